# revision 11
# baseline (speedup 1.0000x reference)
"""Int8 GEMM + per-row requantization on 8 Trainium2 NeuronCores.

Computes: acc = x @ weight_q.T  (int8 x int8 -> int32 exact)
          out = clip(round(acc * (scale_x*scale_w/scale_y)[:, None]), -128, 127).int8
Returns (out, scale_y), matching the reference.

Strategy: row-parallel over seq dim S=8192 -> 8 cores x 1024 rows.
Each core holds its x^T slice [1024k, 1024s] and the full w^T [1024k, 4096o]
in SBUF as bf16 (int8 values are exact in bf16; fp32 PSUM accumulation of
integer products is exact up to 2^24 = 1024*128*128, so the int32 GEMM is
bit-exact). Requant is one fused op per PSUM tile: out_i8 = cvt(psum * r[p])
where the HW fp32->int8 convert rounds-to-nearest-even and saturates --
verified on HW to match jnp.clip(jnp.round(.), -128, 127) exactly.
"""

import sys

try:
    import concourse.bass as bass  # noqa: F401
except ImportError:
    sys.path.insert(0, "/opt/trn_rl_repo")

import numpy as np
import ml_dtypes
from contextlib import ExitStack

import concourse.bass as bass
import concourse.tile as tile
from concourse import bacc, mybir
from concourse.bass_utils import run_bass_kernel_spmd

S, K, O = 8192, 1024, 4096
NCORES = 8
SC = S // NCORES          # 1024 seq rows per core
ST = SC // 128            # 8 s-tiles per core
KT = K // 128             # 8 k chunks
OT = O // 512             # 8 o-tiles of 512

TRACE = False             # test.py can flip this for NTFF profiling
TRACE_DIR = None          # optional dir for trace artifacts
LAST_RESULTS = None       # BassKernelResults of the last run (for test.py)

_CACHE = {}


def _build():
    nc = bacc.Bacc("TRN2", target_bir_lowering=False, debug=False,
                   num_devices=NCORES)

    # partition-major layouts: per SBUF partition p the DRAM data is one
    # contiguous multi-KB run, so DMA descriptors are large and hit
    # near-peak HBM bandwidth.
    xdram = nc.declare_dram_parameter("xt", [128, ST, KT, 128],
                                      mybir.dt.bfloat16, isOutput=False)
    wdram = nc.declare_dram_parameter("wt", [128, OT, KT, 512],
                                      mybir.dt.bfloat16, isOutput=False)
    rdram = nc.declare_dram_parameter("rq", [128, ST],
                                      mybir.dt.float32, isOutput=False)
    odram = nc.declare_dram_parameter("out", [SC, O],
                                      mybir.dt.int8, isOutput=True)

    with tile.TileContext(nc) as tc:
        with ExitStack() as ctx:
            resident = ctx.enter_context(tc.tile_pool(name="resident", bufs=1))
            stage = ctx.enter_context(tc.tile_pool(name="stage", bufs=2))
            psum = ctx.enter_context(tc.tile_pool(name="psum", bufs=4,
                                                  space="PSUM"))

            rsb = resident.tile([128, ST], mybir.dt.float32, tag="rsb")

            xsb = resident.tile([128, ST, KT, 128], mybir.dt.bfloat16,
                                tag="xsb")
            wsb = resident.tile([128, OT, KT, 512], mybir.dt.bfloat16,
                                tag="wsb")

            # Warm the PE's HAM clock gate with dummy matmuls while the
            # first loads are in flight, so the real matmul stream runs
            # at 2.4 GHz from its first instruction.
            warm = resident.tile([128, 512], mybir.dt.bfloat16, tag="warm")
            nc.vector.memset(warm[:], 0.0)
            psum_warm = ctx.enter_context(
                tc.tile_pool(name="psum_warm", bufs=1, space="PSUM"))
            wps = psum_warm.tile([128, 512], mybir.dt.float32, tag="warmps")
            for i in range(10):
                nc.tensor.matmul(wps[:], warm[:, :128], warm[:],
                                 start=(i == 0), stop=(i == 9))

            # Loads ride the two HWDGE rings (cheap launches, FIFO per
            # ring), in consumption order: sync carries the weights
            # o-major (o=0 split in half for an earlier first matmul) so
            # o-group i+1 streams in behind o-group i's matmuls; scalar
            # carries x per-s-tile (s=0 first) so the first matmul is
            # gated only by w[o=0] + x[s=0]. The tiny requant-scale load
            # rides last; it isn't needed until the first requant.
            nc.sync.dma_start(wsb[:, 0, :KT // 2, :],
                              wdram[:, 0, :KT // 2, :])
            nc.sync.dma_start(wsb[:, 0, KT // 2:, :],
                              wdram[:, 0, KT // 2:, :])
            for o in range(1, OT):
                nc.sync.dma_start(wsb[:, o, :, :], wdram[:, o, :, :])
            for j in range(ST):
                nc.scalar.dma_start(xsb[:, j, :, :], xdram[:, j, :, :])
            nc.sync.dma_start(rsb[:], rdram[:])

            for o in range(OT):
                last_o = o == OT - 1
                ot = stage.tile([128, ST, 512], mybir.dt.int8, tag="ostage")
                for j in range(ST):
                    ps = psum.tile([128, 512], mybir.dt.float32, tag="ps")
                    for k in range(KT):
                        nc.tensor.matmul(ps[:], xsb[:, j, k, :],
                                         wsb[:, o, k, :],
                                         start=(k == 0), stop=(k == KT - 1))
                    # fused requant: out = sat_i8(rne(psum * r[p]))
                    if (o + j) % 2 == 0:
                        nc.scalar.activation(
                            ot[:, j, :], ps[:],
                            mybir.ActivationFunctionType.Copy,
                            bias=0.0, scale=rsb[:, j:j + 1])
                    else:
                        nc.vector.tensor_scalar(
                            ot[:, j, :], ps[:], rsb[:, j:j + 1], None,
                            mybir.AluOpType.mult)
                    if last_o:
                        # flush the final o-group per s-tile so the very
                        # last transfer is small (shorter kernel tail)
                        nc.gpsimd.dma_start(
                            odram[j * 128:(j + 1) * 128,
                                  o * 512:(o + 1) * 512],
                            ot[:, j, :])
                if not last_o:
                    nc.gpsimd.dma_start(
                        odram[:, o * 512:(o + 1) * 512]
                        .rearrange("(j p) n -> p j n", p=128),
                        ot[:])

    nc.compile()
    return nc


def kernel(x, weight_q, scale_x, scale_w, scale_y):
    global LAST_RESULTS
    x = np.asarray(x)
    weight_q = np.asarray(weight_q)
    scale_y = np.asarray(scale_y, dtype=np.float32)
    sx = np.float32(np.asarray(scale_x))
    sw = np.float32(np.asarray(scale_w))
    assert x.shape == (S, K) and weight_q.shape == (O, K)

    bf16 = ml_dtypes.bfloat16
    # w^T partition-major: wt[p, o, k, n] = w[o*512+n, k*128+p]
    wt = np.ascontiguousarray(
        weight_q.astype(bf16).reshape(OT, 512, KT, 128).transpose(3, 0, 2, 1))
    # requant scale, fp32 ops in the same order as the reference
    r = (sx * sw) / scale_y  # [S] fp32

    if "nc" not in _CACHE:
        _CACHE["nc"] = _build()
    nc = _CACHE["nc"]

    xb = x.astype(bf16)
    in_maps = []
    for c in range(NCORES):
        xc = xb[c * SC:(c + 1) * SC]  # [1024s, 1024k]
        # x^T partition-major: xt[p, j, k, s] = x_c[j*128+s, k*128+p]
        xt = np.ascontiguousarray(
            xc.reshape(ST, 128, KT, 128).transpose(3, 0, 2, 1))
        rc = np.ascontiguousarray(r[c * SC:(c + 1) * SC].reshape(ST, 128).T)
        in_maps.append({"xt": xt, "wt": wt, "rq": rc})

    LAST_RESULTS = run_bass_kernel_spmd(nc, in_maps, list(range(NCORES)),
                                        trace=TRACE, tmpdir=TRACE_DIR)
    out = np.concatenate(
        [LAST_RESULTS.results[c]["out"] for c in range(NCORES)], axis=0)
    return out, scale_y


# revision 12
# speedup vs baseline: 1.0961x; 1.0961x over previous
"""Int8 GEMM + per-row requantization on 8 Trainium2 NeuronCores.

Computes: acc = x @ weight_q.T  (int8 x int8 -> int32 exact)
          out = clip(round(acc * (scale_x*scale_w/scale_y)[:, None]), -128, 127).int8
Returns (out, scale_y), matching the reference.

Strategy: row-parallel over seq dim S=8192 -> 8 cores x 1024 rows.
Each core holds its x^T slice [1024k, 1024s] and the full w^T [1024k, 4096o]
in SBUF as bf16 (int8 values are exact in bf16; fp32 PSUM accumulation of
integer products is exact up to 2^24 = 1024*128*128, so the int32 GEMM is
bit-exact). Requant is one fused op per PSUM tile: out_i8 = cvt(psum * r[p])
where the HW fp32->int8 convert rounds-to-nearest-even and saturates --
verified on HW to match jnp.clip(jnp.round(.), -128, 127) exactly.
"""

import sys

try:
    import concourse.bass as bass  # noqa: F401
except ImportError:
    sys.path.insert(0, "/opt/trn_rl_repo")

import numpy as np
import ml_dtypes
from contextlib import ExitStack

import concourse.bass as bass
import concourse.tile as tile
from concourse import bacc, mybir
from concourse.bass_utils import run_bass_kernel_spmd

S, K, O = 8192, 1024, 4096
NCORES = 8
SC = S // NCORES          # 1024 seq rows per core
ST = SC // 128            # 8 s-tiles per core
KT = K // 128             # 8 k chunks
OT = O // 512             # 8 o-tiles of 512

TRACE = False             # test.py can flip this for NTFF profiling
TRACE_DIR = None          # optional dir for trace artifacts
LAST_RESULTS = None       # BassKernelResults of the last run (for test.py)

_CACHE = {}


def _build():
    nc = bacc.Bacc("TRN2", target_bir_lowering=False, debug=False,
                   num_devices=NCORES)

    # partition-major layouts: per SBUF partition p the DRAM data is one
    # contiguous multi-KB run, so DMA descriptors are large and hit
    # near-peak HBM bandwidth.
    xdram = nc.declare_dram_parameter("xt", [128, ST, KT, 128],
                                      mybir.dt.bfloat16, isOutput=False)
    wdram = nc.declare_dram_parameter("wt", [128, OT, KT, 512],
                                      mybir.dt.bfloat16, isOutput=False)
    rdram = nc.declare_dram_parameter("rq", [128, ST],
                                      mybir.dt.float32, isOutput=False)
    odram = nc.declare_dram_parameter("out", [SC, O],
                                      mybir.dt.int8, isOutput=True)

    with tile.TileContext(nc) as tc:
        with ExitStack() as ctx:
            resident = ctx.enter_context(tc.tile_pool(name="resident", bufs=1))
            stage = ctx.enter_context(tc.tile_pool(name="stage", bufs=2))
            psum = ctx.enter_context(tc.tile_pool(name="psum", bufs=4,
                                                  space="PSUM"))

            rsb = resident.tile([128, ST], mybir.dt.float32, tag="rsb")

            xsb = resident.tile([128, ST, KT, 128], mybir.dt.bfloat16,
                                tag="xsb")
            wsb = resident.tile([128, OT, KT, 512], mybir.dt.bfloat16,
                                tag="wsb")

            # Warm the PE's HAM clock gate with dummy matmuls while the
            # first loads are in flight, so the real matmul stream runs
            # at 2.4 GHz from its first instruction.
            warm = resident.tile([128, 512], mybir.dt.bfloat16, tag="warm")
            nc.vector.memset(warm[:], 0.0)
            psum_warm = ctx.enter_context(
                tc.tile_pool(name="psum_warm", bufs=1, space="PSUM"))
            wps = psum_warm.tile([128, 512], mybir.dt.float32, tag="warmps")
            for i in range(10):
                nc.tensor.matmul(wps[:], warm[:, :128], warm[:],
                                 start=(i == 0), stop=(i == 9))

            # Loads ride the two HWDGE rings (cheap launches, FIFO per
            # ring) in consumption order, and are batched into at most 8
            # DMAs total so the 8 HWDGE completion lanes never recycle
            # (a 9th in-flight DMA would stall its launch on a lane
            # wait). sync: tiny requant scale, then weights o-major
            # (o=0 split in half for an earlier first matmul, the rest
            # in two big batches). scalar: x s=0, then the rest of x.
            nc.sync.dma_start(rsb[:], rdram[:])
            nc.sync.dma_start(wsb[:, 0, :KT // 2, :],
                              wdram[:, 0, :KT // 2, :])
            nc.sync.dma_start(wsb[:, 0, KT // 2:, :],
                              wdram[:, 0, KT // 2:, :])
            nc.sync.dma_start(wsb[:, 1:4, :, :], wdram[:, 1:4, :, :])
            nc.sync.dma_start(wsb[:, 4:, :, :], wdram[:, 4:, :, :])
            nc.scalar.dma_start(xsb[:, 0, :, :], xdram[:, 0, :, :])
            nc.scalar.dma_start(xsb[:, 1:, :, :], xdram[:, 1:, :, :])

            for o in range(OT):
                last_o = o == OT - 1
                ot = stage.tile([128, ST, 512], mybir.dt.int8, tag="ostage")
                for j in range(ST):
                    ps = psum.tile([128, 512], mybir.dt.float32, tag="ps")
                    for k in range(KT):
                        nc.tensor.matmul(ps[:], xsb[:, j, k, :],
                                         wsb[:, o, k, :],
                                         start=(k == 0), stop=(k == KT - 1))
                    # fused requant: out = sat_i8(rne(psum * r[p]))
                    if (o + j) % 2 == 0:
                        nc.scalar.activation(
                            ot[:, j, :], ps[:],
                            mybir.ActivationFunctionType.Copy,
                            bias=0.0, scale=rsb[:, j:j + 1])
                    else:
                        nc.vector.tensor_scalar(
                            ot[:, j, :], ps[:], rsb[:, j:j + 1], None,
                            mybir.AluOpType.mult)
                    if last_o:
                        # flush the final o-group per s-tile so the very
                        # last transfer is small (shorter kernel tail)
                        nc.gpsimd.dma_start(
                            odram[j * 128:(j + 1) * 128,
                                  o * 512:(o + 1) * 512],
                            ot[:, j, :])
                if not last_o:
                    nc.gpsimd.dma_start(
                        odram[:, o * 512:(o + 1) * 512]
                        .rearrange("(j p) n -> p j n", p=128),
                        ot[:])

    nc.compile()
    return nc


def kernel(x, weight_q, scale_x, scale_w, scale_y):
    global LAST_RESULTS
    x = np.asarray(x)
    weight_q = np.asarray(weight_q)
    scale_y = np.asarray(scale_y, dtype=np.float32)
    sx = np.float32(np.asarray(scale_x))
    sw = np.float32(np.asarray(scale_w))
    assert x.shape == (S, K) and weight_q.shape == (O, K)

    bf16 = ml_dtypes.bfloat16
    # w^T partition-major: wt[p, o, k, n] = w[o*512+n, k*128+p]
    wt = np.ascontiguousarray(
        weight_q.astype(bf16).reshape(OT, 512, KT, 128).transpose(3, 0, 2, 1))
    # requant scale, fp32 ops in the same order as the reference
    r = (sx * sw) / scale_y  # [S] fp32

    if "nc" not in _CACHE:
        _CACHE["nc"] = _build()
    nc = _CACHE["nc"]

    xb = x.astype(bf16)
    in_maps = []
    for c in range(NCORES):
        xc = xb[c * SC:(c + 1) * SC]  # [1024s, 1024k]
        # x^T partition-major: xt[p, j, k, s] = x_c[j*128+s, k*128+p]
        xt = np.ascontiguousarray(
            xc.reshape(ST, 128, KT, 128).transpose(3, 0, 2, 1))
        rc = np.ascontiguousarray(r[c * SC:(c + 1) * SC].reshape(ST, 128).T)
        in_maps.append({"xt": xt, "wt": wt, "rq": rc})

    LAST_RESULTS = run_bass_kernel_spmd(nc, in_maps, list(range(NCORES)),
                                        trace=TRACE, tmpdir=TRACE_DIR)
    out = np.concatenate(
        [LAST_RESULTS.results[c]["out"] for c in range(NCORES)], axis=0)
    return out, scale_y


# revision 13
# speedup vs baseline: 1.1068x; 1.0098x over previous
"""Int8 GEMM + per-row requantization on 8 Trainium2 NeuronCores.

Computes: acc = x @ weight_q.T  (int8 x int8 -> int32 exact)
          out = clip(round(acc * (scale_x*scale_w/scale_y)[:, None]), -128, 127).int8
Returns (out, scale_y), matching the reference.

Strategy: row-parallel over seq dim S=8192 -> 8 cores x 1024 rows.
Each core holds its x^T slice [1024k, 1024s] and the full w^T [1024k, 4096o]
in SBUF as bf16 (int8 values are exact in bf16; fp32 PSUM accumulation of
integer products is exact up to 2^24 = 1024*128*128, so the int32 GEMM is
bit-exact). Requant is one fused op per PSUM tile: out_i8 = cvt(psum * r[p])
where the HW fp32->int8 convert rounds-to-nearest-even and saturates --
verified on HW to match jnp.clip(jnp.round(.), -128, 127) exactly.
"""

import sys

try:
    import concourse.bass as bass  # noqa: F401
except ImportError:
    sys.path.insert(0, "/opt/trn_rl_repo")

import numpy as np
import ml_dtypes
from contextlib import ExitStack

import concourse.bass as bass
import concourse.tile as tile
from concourse import bacc, mybir
from concourse.bass_utils import run_bass_kernel_spmd

S, K, O = 8192, 1024, 4096
NCORES = 8
SC = S // NCORES          # 1024 seq rows per core
ST = SC // 128            # 8 s-tiles per core
KT = K // 128             # 8 k chunks
OT = O // 512             # 8 o-tiles of 512

TRACE = False             # test.py can flip this for NTFF profiling
TRACE_DIR = None          # optional dir for trace artifacts
LAST_RESULTS = None       # BassKernelResults of the last run (for test.py)

_CACHE = {}


def _build():
    nc = bacc.Bacc("TRN2", target_bir_lowering=False, debug=False,
                   num_devices=NCORES)

    # partition-major layouts: per SBUF partition p the DRAM data is one
    # contiguous multi-KB run, so DMA descriptors are large and hit
    # near-peak HBM bandwidth.
    xdram = nc.declare_dram_parameter("xt", [128, ST, KT, 128],
                                      mybir.dt.bfloat16, isOutput=False)
    wdram = nc.declare_dram_parameter("wt", [128, OT, KT, 512],
                                      mybir.dt.bfloat16, isOutput=False)
    rdram = nc.declare_dram_parameter("rq", [128, ST],
                                      mybir.dt.float32, isOutput=False)
    odram = nc.declare_dram_parameter("out", [SC, O],
                                      mybir.dt.int8, isOutput=True)

    with tile.TileContext(nc) as tc:
        with ExitStack() as ctx:
            resident = ctx.enter_context(tc.tile_pool(name="resident", bufs=1))
            stage = ctx.enter_context(tc.tile_pool(name="stage", bufs=2))
            psum = ctx.enter_context(tc.tile_pool(name="psum", bufs=4,
                                                  space="PSUM"))

            rsb = resident.tile([128, ST], mybir.dt.float32, tag="rsb")

            xsb = resident.tile([128, ST, KT, 128], mybir.dt.bfloat16,
                                tag="xsb")
            wsb = resident.tile([128, OT, KT, 512], mybir.dt.bfloat16,
                                tag="wsb")

            # Warm the PE's HAM clock gate with dummy matmuls while the
            # first loads are in flight, so the real matmul stream runs
            # at 2.4 GHz from its first instruction. ~40 matmuls bridge
            # the ~11us from preamble end to first-data arrival (8 cold
            # at ~427ns, then ~213ns warm) without leaving a >3.4us PE
            # idle window that would re-throttle the clock.
            N_WARM = 40
            warm = resident.tile([128, 512], mybir.dt.bfloat16, tag="warm")
            nc.vector.memset(warm[:], 0.0)
            psum_warm = ctx.enter_context(
                tc.tile_pool(name="psum_warm", bufs=1, space="PSUM"))
            wps = psum_warm.tile([128, 512], mybir.dt.float32, tag="warmps")
            for i in range(N_WARM):
                nc.tensor.matmul(wps[:], warm[:, :128], warm[:],
                                 start=(i == 0), stop=(i == N_WARM - 1))

            # Loads ride the two HWDGE rings (cheap launches, FIFO per
            # ring) in consumption order, and are batched into at most 8
            # DMAs total so the 8 HWDGE completion lanes never recycle
            # (a 9th in-flight DMA would stall its launch on a lane
            # wait). sync: tiny requant scale, then weights o-major
            # (o=0 split in half for an earlier first matmul, the rest
            # in two big batches). scalar: x s=0, then the rest of x.
            nc.sync.dma_start(rsb[:], rdram[:])
            nc.sync.dma_start(wsb[:, 0, :KT // 2, :],
                              wdram[:, 0, :KT // 2, :])
            nc.sync.dma_start(wsb[:, 0, KT // 2:, :],
                              wdram[:, 0, KT // 2:, :])
            nc.sync.dma_start(wsb[:, 1:4, :, :], wdram[:, 1:4, :, :])
            nc.sync.dma_start(wsb[:, 4:, :, :], wdram[:, 4:, :, :])
            nc.scalar.dma_start(xsb[:, 0, :, :], xdram[:, 0, :, :])
            nc.scalar.dma_start(xsb[:, 1:, :, :], xdram[:, 1:, :, :])

            for o in range(OT):
                last_o = o == OT - 1
                ot = stage.tile([128, ST, 512], mybir.dt.int8, tag="ostage")
                for j in range(ST):
                    ps = psum.tile([128, 512], mybir.dt.float32, tag="ps")
                    for k in range(KT):
                        nc.tensor.matmul(ps[:], xsb[:, j, k, :],
                                         wsb[:, o, k, :],
                                         start=(k == 0), stop=(k == KT - 1))
                    # fused requant: out = sat_i8(rne(psum * r[p]))
                    if (o + j) % 2 == 0:
                        nc.scalar.activation(
                            ot[:, j, :], ps[:],
                            mybir.ActivationFunctionType.Copy,
                            bias=0.0, scale=rsb[:, j:j + 1])
                    else:
                        nc.vector.tensor_scalar(
                            ot[:, j, :], ps[:], rsb[:, j:j + 1], None,
                            mybir.AluOpType.mult)
                    if last_o:
                        # flush the final o-group per s-tile so the very
                        # last transfer is small (shorter kernel tail)
                        nc.gpsimd.dma_start(
                            odram[j * 128:(j + 1) * 128,
                                  o * 512:(o + 1) * 512],
                            ot[:, j, :])
                if not last_o:
                    nc.gpsimd.dma_start(
                        odram[:, o * 512:(o + 1) * 512]
                        .rearrange("(j p) n -> p j n", p=128),
                        ot[:])

    nc.compile()
    return nc


def kernel(x, weight_q, scale_x, scale_w, scale_y):
    global LAST_RESULTS
    x = np.asarray(x)
    weight_q = np.asarray(weight_q)
    scale_y = np.asarray(scale_y, dtype=np.float32)
    sx = np.float32(np.asarray(scale_x))
    sw = np.float32(np.asarray(scale_w))
    assert x.shape == (S, K) and weight_q.shape == (O, K)

    bf16 = ml_dtypes.bfloat16
    # w^T partition-major: wt[p, o, k, n] = w[o*512+n, k*128+p]
    wt = np.ascontiguousarray(
        weight_q.astype(bf16).reshape(OT, 512, KT, 128).transpose(3, 0, 2, 1))
    # requant scale, fp32 ops in the same order as the reference
    r = (sx * sw) / scale_y  # [S] fp32

    if "nc" not in _CACHE:
        _CACHE["nc"] = _build()
    nc = _CACHE["nc"]

    xb = x.astype(bf16)
    in_maps = []
    for c in range(NCORES):
        xc = xb[c * SC:(c + 1) * SC]  # [1024s, 1024k]
        # x^T partition-major: xt[p, j, k, s] = x_c[j*128+s, k*128+p]
        xt = np.ascontiguousarray(
            xc.reshape(ST, 128, KT, 128).transpose(3, 0, 2, 1))
        rc = np.ascontiguousarray(r[c * SC:(c + 1) * SC].reshape(ST, 128).T)
        in_maps.append({"xt": xt, "wt": wt, "rq": rc})

    LAST_RESULTS = run_bass_kernel_spmd(nc, in_maps, list(range(NCORES)),
                                        trace=TRACE, tmpdir=TRACE_DIR)
    out = np.concatenate(
        [LAST_RESULTS.results[c]["out"] for c in range(NCORES)], axis=0)
    return out, scale_y


# revision 14
# speedup vs baseline: 1.1355x; 1.0259x over previous
"""Int8 GEMM + per-row requantization on 8 Trainium2 NeuronCores.

Computes: acc = x @ weight_q.T  (int8 x int8 -> int32 exact)
          out = clip(round(acc * (scale_x*scale_w/scale_y)[:, None]), -128, 127).int8
Returns (out, scale_y), matching the reference.

Strategy: row-parallel over seq dim S=8192 -> 8 cores x 1024 rows.
Each core holds its x^T slice [1024k, 1024s] and the full w^T [1024k, 4096o]
in SBUF as bf16 (int8 values are exact in bf16; fp32 PSUM accumulation of
integer products is exact up to 2^24 = 1024*128*128, so the int32 GEMM is
bit-exact). Inputs are DMA'd as int8 (half the HBM traffic) and upcast to
bf16 on the otherwise-idle vector/scalar engines. Requant is one fused op
per PSUM tile: out_i8 = cvt(psum * r[p]) where the HW fp32->int8 convert
rounds-to-nearest-even and saturates -- verified on HW to match
jnp.clip(jnp.round(.), -128, 127) exactly.
"""

import sys

try:
    import concourse.bass as bass  # noqa: F401
except ImportError:
    sys.path.insert(0, "/opt/trn_rl_repo")

import numpy as np
from contextlib import ExitStack

import concourse.bass as bass
import concourse.tile as tile
from concourse import bacc, mybir
from concourse.bass_utils import run_bass_kernel_spmd

S, K, O = 8192, 1024, 4096
NCORES = 8
SC = S // NCORES          # 1024 seq rows per core
ST = SC // 128            # 8 s-tiles per core
KT = K // 128             # 8 k chunks
OT = O // 512             # 8 o-tiles of 512

TRACE = False             # test.py can flip this for NTFF profiling
TRACE_DIR = None          # optional dir for trace artifacts
LAST_RESULTS = None       # BassKernelResults of the last run (for test.py)

_CACHE = {}


def _build():
    nc = bacc.Bacc("TRN2", target_bir_lowering=False, debug=False,
                   num_devices=NCORES)

    # partition-major int8 layouts: per SBUF partition p the DRAM data is
    # one contiguous multi-KB run, so DMA descriptors are large and hit
    # near-peak HBM bandwidth.
    xdram = nc.declare_dram_parameter("xt", [128, ST, KT, 128],
                                      mybir.dt.int8, isOutput=False)
    wdram = nc.declare_dram_parameter("wt", [128, OT, KT, 512],
                                      mybir.dt.int8, isOutput=False)
    rdram = nc.declare_dram_parameter("rq", [128, ST],
                                      mybir.dt.float32, isOutput=False)
    odram = nc.declare_dram_parameter("out", [SC, O],
                                      mybir.dt.int8, isOutput=True)

    with tile.TileContext(nc) as tc:
        with ExitStack() as ctx:
            resident = ctx.enter_context(tc.tile_pool(name="resident", bufs=1))
            stage = ctx.enter_context(tc.tile_pool(name="stage", bufs=2))
            psum = ctx.enter_context(tc.tile_pool(name="psum", bufs=4,
                                                  space="PSUM"))

            rsb = resident.tile([128, ST], mybir.dt.float32, tag="rsb")

            xsb8 = resident.tile([128, ST, KT, 128], mybir.dt.int8,
                                 tag="xsb8")
            wsb8 = resident.tile([128, OT, KT, 512], mybir.dt.int8,
                                 tag="wsb8")
            xsb = resident.tile([128, ST, KT, 128], mybir.dt.bfloat16,
                                tag="xsb")
            wsb = resident.tile([128, OT, KT, 512], mybir.dt.bfloat16,
                                tag="wsb")

            # Warm the PE's HAM clock gate with dummy matmuls while the
            # first loads are in flight, so the real matmul stream runs
            # at 2.4 GHz from its first instruction (8 cold at ~427ns,
            # then ~213ns each, bridging to first-data-ready).
            N_WARM = 20
            warm = resident.tile([128, 512], mybir.dt.bfloat16, tag="warm")
            nc.vector.memset(warm[:], 0.0)
            psum_warm = ctx.enter_context(
                tc.tile_pool(name="psum_warm", bufs=1, space="PSUM"))
            wps = psum_warm.tile([128, 512], mybir.dt.float32, tag="warmps")
            for i in range(N_WARM):
                nc.tensor.matmul(wps[:], warm[:, :128], warm[:],
                                 start=(i == 0), stop=(i == N_WARM - 1))

            # Loads ride the two HWDGE rings (cheap launches, FIFO per
            # ring) in consumption order, batched into at most 8 DMAs so
            # the 8 HWDGE completion lanes never recycle. sync: tiny
            # requant scale, then weights o-major (o=0 alone for an
            # early first matmul, the rest in two batches). scalar:
            # x s=0 first, then the rest of x.
            nc.sync.dma_start(rsb[:], rdram[:])
            nc.sync.dma_start(wsb8[:, 0, :, :], wdram[:, 0, :, :])
            nc.sync.dma_start(wsb8[:, 1:4, :, :], wdram[:, 1:4, :, :])
            nc.sync.dma_start(wsb8[:, 4:, :, :], wdram[:, 4:, :, :])
            nc.scalar.dma_start(xsb8[:, 0, :, :], xdram[:, 0, :, :])
            nc.scalar.dma_start(xsb8[:, 1:, :, :], xdram[:, 1:, :, :])

            # int8 -> bf16 upcasts on the vector/scalar engines. x per
            # s-tile on scalar (small), w per o-tile on vector, emitted
            # in consumption order so engine-FIFO order matches data
            # arrival and never blocks later requants.
            for j in range(ST):
                nc.scalar.activation(xsb[:, j, :, :], xsb8[:, j, :, :],
                                     mybir.ActivationFunctionType.Copy,
                                     bias=0.0, scale=1.0)
            nc.vector.tensor_copy(wsb[:, 0, :, :], wsb8[:, 0, :, :])

            for o in range(OT):
                last_o = o == OT - 1
                if o + 1 < OT:
                    nc.vector.tensor_copy(wsb[:, o + 1, :, :],
                                          wsb8[:, o + 1, :, :])
                ot = stage.tile([128, ST, 512], mybir.dt.int8, tag="ostage")
                for j in range(ST):
                    ps = psum.tile([128, 512], mybir.dt.float32, tag="ps")
                    for k in range(KT):
                        nc.tensor.matmul(ps[:], xsb[:, j, k, :],
                                         wsb[:, o, k, :],
                                         start=(k == 0), stop=(k == KT - 1))
                    # fused requant: out = sat_i8(rne(psum * r[p]))
                    if (o + j) % 2 == 0:
                        nc.scalar.activation(
                            ot[:, j, :], ps[:],
                            mybir.ActivationFunctionType.Copy,
                            bias=0.0, scale=rsb[:, j:j + 1])
                    else:
                        nc.vector.tensor_scalar(
                            ot[:, j, :], ps[:], rsb[:, j:j + 1], None,
                            mybir.AluOpType.mult)
                    if last_o:
                        # flush the final o-group per s-tile so the very
                        # last transfer is small (shorter kernel tail)
                        nc.gpsimd.dma_start(
                            odram[j * 128:(j + 1) * 128,
                                  o * 512:(o + 1) * 512],
                            ot[:, j, :])
                if not last_o:
                    nc.gpsimd.dma_start(
                        odram[:, o * 512:(o + 1) * 512]
                        .rearrange("(j p) n -> p j n", p=128),
                        ot[:])

    nc.compile()
    return nc


def kernel(x, weight_q, scale_x, scale_w, scale_y):
    global LAST_RESULTS
    x = np.asarray(x)
    weight_q = np.asarray(weight_q)
    scale_y = np.asarray(scale_y, dtype=np.float32)
    sx = np.float32(np.asarray(scale_x))
    sw = np.float32(np.asarray(scale_w))
    assert x.shape == (S, K) and weight_q.shape == (O, K)

    i8 = np.int8
    # w^T partition-major int8: wt[p, o, k, n] = w[o*512+n, k*128+p]
    wt = np.ascontiguousarray(
        weight_q.astype(i8).reshape(OT, 512, KT, 128).transpose(3, 0, 2, 1))
    # requant scale, fp32 ops in the same order as the reference
    r = (sx * sw) / scale_y  # [S] fp32

    if "nc" not in _CACHE:
        _CACHE["nc"] = _build()
    nc = _CACHE["nc"]

    xb = x.astype(i8)
    in_maps = []
    for c in range(NCORES):
        xc = xb[c * SC:(c + 1) * SC]  # [1024s, 1024k]
        # x^T partition-major int8: xt[p, j, k, s] = x_c[j*128+s, k*128+p]
        xt = np.ascontiguousarray(
            xc.reshape(ST, 128, KT, 128).transpose(3, 0, 2, 1))
        rc = np.ascontiguousarray(r[c * SC:(c + 1) * SC].reshape(ST, 128).T)
        in_maps.append({"xt": xt, "wt": wt, "rq": rc})

    LAST_RESULTS = run_bass_kernel_spmd(nc, in_maps, list(range(NCORES)),
                                        trace=TRACE, tmpdir=TRACE_DIR)
    out = np.concatenate(
        [LAST_RESULTS.results[c]["out"] for c in range(NCORES)], axis=0)
    return out, scale_y
